# revision 16
# baseline (speedup 1.0000x reference)
"""Trainium2 Bass kernel for a GPT-style decoder block (b=4, t=2048, d=1024, 16 heads).

Sharding: 8 cores, collective-free. Core c handles batch c//2 and sequence half
c%2 (1024 query rows). K/V are computed for the full 2048-row sequence on both
cores of a batch pair (duplicated compute instead of collectives). The per-core
program is identical (SPMD); per-core behavior differs only through input
values: x is passed with the core's own half first (kv order permuted), and the
causal mask (transposed, multiplicative) is built host-side per core.

Dataflow on each core (bf16 matmuls, fp32 accumulation / LN / softmax denoms):
  LN1(x full)->H -> PE-transpose -> HT[d, kv]
  KT[1024,2048]=Wk^T@HT, QT[1024,1024]=Wq^T@HT[:, :1024], V'[kv, 16*(64+1)]
  per head: S^T[kv,q] = K Q^T (PE), P^T = exp(S^T/8)*maskT, Y'^T: col-major PV
    with an extra ones column in V' giving softmax denominators, normalize via
    reciprocal + partition-broadcast
  o_proj + residual -> LN2 -> H2T -> fc1+gelu (G^T layout) -> fc2 + residual
"""

import os
import sys

import numpy as np
import ml_dtypes

for _p in ("/opt/trn_rl_repo",):
    if _p not in sys.path and os.path.isdir(_p):
        sys.path.insert(0, _p)

import concourse.bass as bass
import concourse.tile as tile
from concourse import mybir
from concourse.bass_utils import run_bass_kernel_spmd
from concourse.masks import make_identity
from concourse.vector_clock import ScopedClock


class SplitTailTileContext(tile.TileContext):
    """TileContext whose tail drain splits its semaphore waits across NOPs.

    The stock epilogue attaches every outstanding semaphore wait to a single
    Drain; walrus's CTRL codegen rejects >N waits per instruction
    ("Too many sync wait commands"). Carry one wait per NOP instead.
    """

    def _drain_and_barrier(self, tick_clock, wait_clock):
        nc = self.nc
        collector = nc.sync.nop(nofuse=True)
        wait_clock.add_sem_waits(
            collector.ins, ScopedClock({None: tick_clock.global_clock})
        )
        si = collector.ins.sync_info
        waits = list(si.on_wait or []) if si is not None else []
        if si is not None:
            si.on_wait = waits[:1]
        for w in waits[1:]:
            n = nc.sync.nop(nofuse=True)
            if n.ins.sync_info is None:
                n.ins.sync_info = mybir.SyncInfo(on_wait=[w], on_update=[])
            else:
                n.ins.sync_info.on_wait = [w]
        nc.sync.drain()
        nc.all_engine_barrier()
        assert self.sems is not None
        popped = nc._tile_sem_poison_stack.pop()
        assert popped is self._sem_poison
        nc.clear_and_free_semaphores(list(self.sems.allocated().values()))
        nc.all_engine_barrier()


F32 = mybir.dt.float32
BF16 = mybir.dt.bfloat16
AF = mybir.ActivationFunctionType

B, T, D = 4, 2048, 1024
NH, HD = 16, 64
FF = 4 * D
TQ = T // 2          # own query rows per core
EPS = 1e-5
N_CORES = 8

KV_TILES = T // 128          # 16
Q_TILES = TQ // 128          # 8
D_TILES = D // 128           # 8
FF_TILES = FF // 128         # 32
VW = HD + 1                  # V' per-head width (64 + ones column)


def _bcast_ap(ap, parts):
    """DRAM [N] -> [parts, N] partition-broadcast access pattern."""
    return bass.AP(tensor=ap.tensor, offset=ap.offset, ap=[[0, parts]] + list(ap.ap))


def _col_ap(ap, n):
    """DRAM [n*128] -> SBUF-shaped [128, n] AP (element (p, t) = src[t*128+p])."""
    return bass.AP(tensor=ap.tensor, offset=ap.offset, ap=[[1, 128], [128, n]])


def _split_excess_waits(nc, limit=1):
    """Hoist surplus semaphore waits onto same-engine NOPs before each inst.

    The neuronxcc here rejects instructions carrying more than a couple of
    sync waits ("Too many sync wait commands"), so cap every instruction at
    `limit` waits and carry the rest on preceding NOPs (sequencers execute
    their stream in order, so the ordering semantics are identical).
    """
    for fn in nc.m.functions:
        for bb in fn.blocks:
            new_insts = []
            for inst in bb.instructions:
                si = inst.sync_info
                if si is not None and si.on_wait and len(si.on_wait) > limit:
                    waits = list(si.on_wait)
                    keep = waits[-limit:]
                    for w in waits[:-limit]:
                        nop = mybir.InstNoOp(
                            name=f"I-{nc.next_id()}", ins=[], outs=[])
                        nop.engine = inst.engine
                        nop.sync_info = mybir.SyncInfo(on_wait=[w],
                                                       on_update=[])
                        nc.register_instruction(nop, overwrite=True)
                        new_insts.append(nop)
                    si.on_wait = keep
                new_insts.append(inst)
            bb.instructions[:] = new_insts


def build_program(gelu=True):
    nc = bass.Bass()

    x_h = nc.declare_dram_parameter("x", [T, D], F32, isOutput=False)
    maskT_h = nc.declare_dram_parameter("maskT", [T, TQ], BF16, isOutput=False)
    wqkv_h = nc.declare_dram_parameter("wqkv", [D, 3 * D], BF16, isOutput=False)
    wo_h = nc.declare_dram_parameter("wo", [D, D], BF16, isOutput=False)
    w1_h = nc.declare_dram_parameter("w1", [D, FF], BF16, isOutput=False)
    w2_h = nc.declare_dram_parameter("w2", [FF, D], BF16, isOutput=False)
    bq_h = nc.declare_dram_parameter("bq", [D], F32, isOutput=False)
    bk_h = nc.declare_dram_parameter("bk", [D], F32, isOutput=False)
    bv_h = nc.declare_dram_parameter("bv", [D], BF16, isOutput=False)
    bo_h = nc.declare_dram_parameter("bo", [D], BF16, isOutput=False)
    b1_h = nc.declare_dram_parameter("b1", [FF], F32, isOutput=False)
    b2_h = nc.declare_dram_parameter("b2", [D], BF16, isOutput=False)
    g1_h = nc.declare_dram_parameter("g1", [D], F32, isOutput=False)
    be1_h = nc.declare_dram_parameter("be1", [D], F32, isOutput=False)
    g2_h = nc.declare_dram_parameter("g2", [D], F32, isOutput=False)
    be2_h = nc.declare_dram_parameter("be2", [D], F32, isOutput=False)
    y_h = nc.declare_dram_parameter("y", [TQ, D], F32, isOutput=True)

    with SplitTailTileContext(nc) as tc:
        with (
            tc.tile_pool(name="consts", bufs=1) as consts,
            tc.tile_pool(name="ytpool", bufs=1) as ytpool,
        ):
            # --- constants ---
            ident = consts.tile([128, 128], BF16)
            make_identity(nc, ident[:])
            ones1 = consts.tile([1, 128], BF16)
            nc.vector.memset(ones1[:], 1.0)
            eps_sb = consts.tile([128, 1], F32)
            nc.vector.memset(eps_sb[:], EPS)
            ones64 = consts.tile([1, 64], F32)
            nc.vector.memset(ones64[:], 1.0)
            g1_sb = consts.tile([128, D], F32)
            nc.sync.dma_start(out=g1_sb[:], in_=_bcast_ap(g1_h[:], 128))
            be1_sb = consts.tile([128, D], F32)
            nc.sync.dma_start(out=be1_sb[:], in_=_bcast_ap(be1_h[:], 128))
            g2_sb = consts.tile([128, D], F32)
            nc.sync.dma_start(out=g2_sb[:], in_=_bcast_ap(g2_h[:], 128))
            be2_sb = consts.tile([128, D], F32)
            nc.sync.dma_start(out=be2_sb[:], in_=_bcast_ap(be2_h[:], 128))
            bq_sb = consts.tile([128, D_TILES], F32)
            nc.sync.dma_start(out=bq_sb[:], in_=_col_ap(bq_h[:], D_TILES))
            bk_sb = consts.tile([128, D_TILES], F32)
            nc.sync.dma_start(out=bk_sb[:], in_=_col_ap(bk_h[:], D_TILES))
            b1_sb = consts.tile([128, FF_TILES], F32)
            nc.sync.dma_start(out=b1_sb[:], in_=_col_ap(b1_h[:], FF_TILES))
            bv_sb = consts.tile([1, D], BF16)
            nc.sync.dma_start(out=bv_sb[:], in_=bv_h[:][None, :])
            bo_sb = consts.tile([1, D], BF16)
            nc.sync.dma_start(out=bo_sb[:], in_=bo_h[:][None, :])
            b2_sb = consts.tile([1, D], BF16)
            nc.sync.dma_start(out=b2_sb[:], in_=b2_h[:][None, :])

            # attention output, transposed/stacked [dv, q]; lives P3->P4
            YT = [ytpool.tile([128, TQ], BF16, name=f"YT{i}", tag=f"YT{i}")
                  for i in range(D_TILES)]

            def layernorm(dst_bf16, src_f32, g_sb, be_sb, pool):
                """dst = LN(src) * g + be, row-wise over free dim D."""
                stats = pool.tile([128, 2, 6], F32, tag="ln_stats")
                mv = pool.tile([128, 2], F32, tag="ln_mv")
                src3 = src_f32.rearrange("p (s f) -> p s f", s=2)
                for s in range(2):
                    nc.vector.bn_stats(out=stats[:, s, :], in_=src3[:, s, :])
                nc.vector.bn_aggr(out=mv[:], in_=stats[:])
                rstd = pool.tile([128, 1], F32, tag="ln_rstd")
                nc.scalar.activation(out=rstd[:], in_=mv[:, 1:2], func=AF.Sqrt,
                                     bias=eps_sb[:], scale=1.0)
                nc.vector.reciprocal(out=rstd[:], in_=rstd[:])
                t1 = pool.tile([128, D], F32, tag="ln_t1")
                nc.vector.tensor_scalar(out=t1[:], in0=src_f32,
                                        scalar1=mv[:, 0:1], scalar2=rstd[:],
                                        op0=mybir.AluOpType.subtract,
                                        op1=mybir.AluOpType.mult)
                nc.vector.tensor_mul(out=t1[:], in0=t1[:], in1=g_sb[:])
                nc.vector.tensor_add(out=dst_bf16, in0=t1[:], in1=be_sb[:])

            with tc.tile_pool(name="htpool", bufs=1) as htpool:
                HT = [htpool.tile([128, T], BF16, name=f"HT{i}", tag=f"HT{i}")
                      for i in range(D_TILES)]
                # maskT reuses HT's storage in phase 3 (disjoint lifetimes;
                # Tile serializes the overwrite on the WAR dependency).
                maskT = [HT[t // 2][:, (t % 2) * TQ:(t % 2 + 1) * TQ]
                         for t in range(KV_TILES)]

                # --- phase 1: LN1(x) -> H -> transpose -> HT ---
                with (
                    tc.tile_pool(name="p1x", bufs=3) as p1x,
                    tc.tile_pool(name="p1h", bufs=6) as p1h,
                    tc.tile_pool(name="p1tmp", bufs=4) as p1tmp,
                    tc.tile_pool(name="p1ps", bufs=4, space="PSUM") as p1ps,
                ):
                    for g in range(KV_TILES // 4):
                        hts = []
                        for qq in range(4):
                            qt = g * 4 + qq
                            xt = p1x.tile([128, D], F32, tag="xt")
                            nc.sync.dma_start(out=xt[:],
                                              in_=x_h[qt * 128:(qt + 1) * 128, :])
                            ht = p1h.tile([128, D], BF16, tag="h")
                            layernorm(ht[:], xt[:], g1_sb, be1_sb, p1tmp)
                            hts.append(ht)
                        for j in range(D_TILES):
                            ps = p1ps.tile([128, 512], BF16, tag="tps")
                            for qq in range(4):
                                nc.tensor.transpose(
                                    ps[:, qq * 128:(qq + 1) * 128],
                                    hts[qq][:, j * 128:(j + 1) * 128],
                                    ident[:])
                            nc.scalar.copy(out=HT[j][:, g * 512:(g + 1) * 512],
                                           in_=ps[:])

                with tc.tile_pool(name="kqv", bufs=1) as kqv:
                    KT = [kqv.tile([128, T], BF16, name=f"KT{i}", tag=f"KT{i}")
                          for i in range(D_TILES)]
                    QT = [kqv.tile([128, TQ], BF16, name=f"QT{i}", tag=f"QT{i}")
                          for i in range(D_TILES)]
                    Vp = [kqv.tile([128, NH * VW], BF16, name=f"Vp{i}",
                                   tag=f"Vp{i}")
                          for i in range(KV_TILES)]

                    # --- phase 2: projections KT, QT, V' ---
                    with (
                        tc.tile_pool(name="p2w", bufs=1) as p2w,
                        tc.tile_pool(name="p2ps", bufs=4, space="PSUM") as p2ps,
                    ):
                        wqkv_sb = [p2w.tile([128, 3 * D], BF16, name=f"wqkv{k}",
                                            tag=f"wqkv{k}")
                                   for k in range(D_TILES)]
                        for k in range(D_TILES):
                            nc.sync.dma_start(out=wqkv_sb[k][:],
                                              in_=wqkv_h[k * 128:(k + 1) * 128, :])

                        for mo in range(D_TILES):
                            for c in range(T // 512):
                                ps = p2ps.tile([128, 512], F32, tag="proj")
                                for k in range(D_TILES):
                                    nc.tensor.matmul(
                                        ps[:],
                                        wqkv_sb[k][:, D + mo * 128:
                                                   D + (mo + 1) * 128],
                                        HT[k][:, c * 512:(c + 1) * 512],
                                        start=(k == 0), stop=(k == D_TILES - 1))
                                nc.scalar.activation(
                                    out=KT[mo][:, c * 512:(c + 1) * 512],
                                    in_=ps[:], func=AF.Identity,
                                    bias=bk_sb[:, mo:mo + 1], scale=1.0)
                        for mo in range(D_TILES):
                            for c in range(TQ // 512):
                                ps = p2ps.tile([128, 512], F32, tag="proj")
                                for k in range(D_TILES):
                                    nc.tensor.matmul(
                                        ps[:],
                                        wqkv_sb[k][:, mo * 128:(mo + 1) * 128],
                                        HT[k][:, c * 512:(c + 1) * 512],
                                        start=(k == 0), stop=(k == D_TILES - 1))
                                nc.scalar.activation(
                                    out=QT[mo][:, c * 512:(c + 1) * 512],
                                    in_=ps[:], func=AF.Identity,
                                    bias=bq_sb[:, mo:mo + 1], scale=1.0)
                        for t in range(KV_TILES):
                            vv = Vp[t][:].rearrange("p (h w) -> p h w", w=VW)
                            nc.vector.memset(vv[:, :, HD:HD + 1], 1.0)
                            for c in range(D // 512):
                                ps = p2ps.tile([128, 512], F32, tag="proj")
                                for k in range(D_TILES):
                                    nc.tensor.matmul(
                                        ps[:],
                                        HT[k][:, t * 128:(t + 1) * 128],
                                        wqkv_sb[k][:, 2 * D + c * 512:
                                                   2 * D + (c + 1) * 512],
                                        start=(k == 0), stop=False)
                                nc.tensor.matmul(
                                    ps[:], ones1[:],
                                    bv_sb[:, c * 512:(c + 1) * 512],
                                    start=False, stop=True)
                                nc.vector.tensor_copy(
                                    out=vv[:, c * 8:(c + 1) * 8, 0:HD],
                                    in_=ps[:].rearrange("p (h e) -> p h e", e=HD))

                    # --- phase 3: attention (maskT overwrites HT storage) ---
                    with (
                        tc.tile_pool(name="p3pt", bufs=KV_TILES + 2) as p3pt,
                        tc.tile_pool(name="p3sm", bufs=2) as p3sm,
                        tc.tile_pool(name="p3st", bufs=2, space="PSUM") as p3st,
                        tc.tile_pool(name="p3yp", bufs=2, space="PSUM") as p3yp,
                        tc.tile_pool(name="p3rb", bufs=2, space="PSUM") as p3rb,
                    ):
                        for t in range(KV_TILES):
                            nc.sync.dma_start(out=maskT[t],
                                              in_=maskT_h[t * 128:(t + 1) * 128, :])

                        for h in range(NH):
                            it = h // 2
                            pr = (h % 2) * 64
                            pts = []
                            for t in range(KV_TILES):
                                st = p3st.tile([128, TQ], F32, tag="st")
                                for c in range(TQ // 512):
                                    nc.tensor.matmul(
                                        st[:, c * 512:(c + 1) * 512],
                                        KT[it][pr:pr + 64, t * 128:(t + 1) * 128],
                                        QT[it][pr:pr + 64, c * 512:(c + 1) * 512],
                                        start=True, stop=True)
                                pt = p3pt.tile([128, TQ], BF16, tag="pt")
                                nc.scalar.activation(out=pt[:], in_=st[:],
                                                     func=AF.Exp,
                                                     scale=1.0 / np.sqrt(HD))
                                nc.vector.tensor_mul(out=pt[:], in0=pt[:],
                                                     in1=maskT[t])
                                pts.append(pt)
                            r_sb = p3sm.tile([1, TQ], F32, tag="r")
                            yps = []
                            for c in range(TQ // 512):
                                yp = p3yp.tile([VW, 512], F32, tag="yp")
                                for t in range(KV_TILES):
                                    nc.tensor.matmul(
                                        yp[:],
                                        Vp[t][:, h * VW:(h + 1) * VW],
                                        pts[t][:, c * 512:(c + 1) * 512],
                                        start=(t == 0), stop=(t == KV_TILES - 1))
                                nc.vector.reciprocal(
                                    out=r_sb[:, c * 512:(c + 1) * 512],
                                    in_=yp[HD:VW, :])
                                yps.append(yp)
                            # broadcast 1/sums across 64 partitions via a
                            # K=1 PE matmul (ones column x row vector)
                            rb = p3sm.tile([64, TQ], F32, tag="rb")
                            for c in range(TQ // 512):
                                rb_ps = p3rb.tile([64, 512], F32, tag="rbps")
                                nc.tensor.matmul(
                                    rb_ps[:], ones64[:],
                                    r_sb[:, c * 512:(c + 1) * 512],
                                    start=True, stop=True)
                                nc.scalar.copy(
                                    out=rb[:, c * 512:(c + 1) * 512],
                                    in_=rb_ps[:])
                            for c in range(TQ // 512):
                                nc.vector.tensor_mul(
                                    out=YT[it][pr:pr + 64, c * 512:(c + 1) * 512],
                                    in0=yps[c][0:HD, :],
                                    in1=rb[:, c * 512:(c + 1) * 512])

            # HT/KQV/PT storage freed here
            with tc.tile_pool(name="x2pool", bufs=1) as x2pool:
                x2 = [x2pool.tile([128, D], F32, name=f"x2{i}", tag=f"x2{i}")
                      for i in range(Q_TILES)]

                # --- phase 4: o_proj + residual ---
                with (
                    tc.tile_pool(name="p4w", bufs=1) as p4w,
                    tc.tile_pool(name="p4x", bufs=3) as p4x,
                    tc.tile_pool(name="p4ps", bufs=4, space="PSUM") as p4ps,
                ):
                    wo_sb = [p4w.tile([128, D], BF16, name=f"wo{k}", tag=f"wo{k}")
                             for k in range(D_TILES)]
                    for k in range(D_TILES):
                        nc.sync.dma_start(out=wo_sb[k][:],
                                          in_=wo_h[k * 128:(k + 1) * 128, :])
                    for m in range(Q_TILES):
                        xr = p4x.tile([128, D], F32, tag="xr")
                        nc.sync.dma_start(out=xr[:],
                                          in_=x_h[m * 128:(m + 1) * 128, :])
                        for c in range(D // 512):
                            ps = p4ps.tile([128, 512], F32, tag="ops")
                            for k in range(D_TILES):
                                nc.tensor.matmul(
                                    ps[:],
                                    YT[k][:, m * 128:(m + 1) * 128],
                                    wo_sb[k][:, c * 512:(c + 1) * 512],
                                    start=(k == 0), stop=False)
                            nc.tensor.matmul(ps[:], ones1[:],
                                             bo_sb[:, c * 512:(c + 1) * 512],
                                             start=False, stop=True)
                            nc.vector.tensor_add(
                                out=x2[m][:, c * 512:(c + 1) * 512],
                                in0=ps[:], in1=xr[:, c * 512:(c + 1) * 512])

                # --- phase 5: LN2 -> H2 -> H2T ---
                with tc.tile_pool(name="h2tpool", bufs=1) as h2tpool:
                    H2T = [h2tpool.tile([128, TQ], BF16, name=f"H2T{i}",
                                        tag=f"H2T{i}")
                           for i in range(D_TILES)]
                    with (
                        tc.tile_pool(name="p5h", bufs=6) as p5h,
                        tc.tile_pool(name="p5tmp", bufs=4) as p5tmp,
                        tc.tile_pool(name="p5ps", bufs=4, space="PSUM") as p5ps,
                    ):
                        for g in range(Q_TILES // 4):
                            hts = []
                            for qq in range(4):
                                m = g * 4 + qq
                                ht = p5h.tile([128, D], BF16, tag="h2")
                                layernorm(ht[:], x2[m][:], g2_sb, be2_sb, p5tmp)
                                hts.append(ht)
                            for j in range(D_TILES):
                                ps = p5ps.tile([128, 512], BF16, tag="tps")
                                for qq in range(4):
                                    nc.tensor.transpose(
                                        ps[:, qq * 128:(qq + 1) * 128],
                                        hts[qq][:, j * 128:(j + 1) * 128],
                                        ident[:])
                                nc.scalar.copy(
                                    out=H2T[j][:, g * 512:(g + 1) * 512],
                                    in_=ps[:])

                    # --- phase 6: fc1 + gelu -> GT ---
                    with (
                        tc.tile_pool(name="p6g", bufs=1) as p6g,
                        tc.tile_pool(name="p6w", bufs=3) as p6w,
                        tc.tile_pool(name="p6ps", bufs=2, space="PSUM") as p6ps,
                    ):
                        GT = [p6g.tile([128, TQ], BF16, name=f"GT{i}",
                                       tag=f"GT{i}")
                              for i in range(FF_TILES)]
                        for mf in range(FF_TILES):
                            w1t = p6w.tile([128, D_TILES, 128], BF16, tag="w1t")
                            nc.sync.dma_start(
                                out=w1t[:],
                                in_=w1_h[:, mf * 128:(mf + 1) * 128].rearrange(
                                    "(k p) m -> p k m", p=128))
                            ps = p6ps.tile([128, TQ], F32, tag="fc1")
                            for c in range(TQ // 512):
                                for k in range(D_TILES):
                                    nc.tensor.matmul(
                                        ps[:, c * 512:(c + 1) * 512],
                                        w1t[:, k, :],
                                        H2T[k][:, c * 512:(c + 1) * 512],
                                        start=(k == 0), stop=(k == D_TILES - 1))
                            nc.scalar.activation(out=GT[mf][:], in_=ps[:],
                                                 func=(AF.Gelu if gelu
                                                       else AF.Identity),
                                                 bias=b1_sb[:, mf:mf + 1],
                                                 scale=1.0)

                        # --- phase 7: fc2 + residual -> y ---
                        with (
                            tc.tile_pool(name="p7w", bufs=1) as p7w,
                            tc.tile_pool(name="p7y", bufs=3) as p7y,
                            tc.tile_pool(name="p7ps", bufs=4,
                                         space="PSUM") as p7ps,
                        ):
                            for half in range(2):
                                w2h = p7w.tile([128, FF_TILES, 512], BF16,
                                               tag="w2h")
                                nc.sync.dma_start(
                                    out=w2h[:],
                                    in_=w2_h[:, half * 512:(half + 1) * 512]
                                    .rearrange("(k p) n -> p k n", p=128))
                                for m in range(Q_TILES):
                                    ps = p7ps.tile([128, 512], F32, tag="fc2")
                                    for k in range(FF_TILES):
                                        nc.tensor.matmul(
                                            ps[:],
                                            GT[k][:, m * 128:(m + 1) * 128],
                                            w2h[:, k, :],
                                            start=(k == 0), stop=False)
                                    nc.tensor.matmul(
                                        ps[:], ones1[:],
                                        b2_sb[:, half * 512:(half + 1) * 512],
                                        start=False, stop=True)
                                    yt = p7y.tile([128, 512], F32, tag="yt")
                                    nc.vector.tensor_add(
                                        out=yt[:], in0=ps[:],
                                        in1=x2[m][:, half * 512:(half + 1) * 512])
                                    nc.sync.dma_start(
                                        out=y_h[m * 128:(m + 1) * 128,
                                                half * 512:(half + 1) * 512],
                                        in_=yt[:])
    _split_excess_waits(nc)
    return nc


_prog_cache = {}


def _get_program():
    if "nc" not in _prog_cache:
        _prog_cache["nc"] = build_program()
    return _prog_cache["nc"]


def _make_maskT(h):
    """[T, TQ] bf16 multiplicative mask in permuted-kv x own-q coordinates."""
    j = np.arange(T)[:, None]     # kv index in permuted order
    i = np.arange(TQ)[None, :]    # own q index
    if h == 0:
        keep = j <= i
    else:
        keep = (j >= TQ) | (j <= i)
    return keep.astype(ml_dtypes.bfloat16)


def _prepare_in_maps(x, w_qkv, b_qkv, w_o, b_o, g1, be1, g2, be2,
                     w1, b1, w2, b2):
    x = np.asarray(x, np.float32)
    bf = lambda a: np.asarray(a, np.float32).astype(ml_dtypes.bfloat16)
    f32 = lambda a: np.asarray(a, np.float32)

    shared = dict(
        wqkv=bf(w_qkv), wo=bf(w_o), w1=bf(w1), w2=bf(w2),
        bq=f32(b_qkv[:D]), bk=f32(b_qkv[D:2 * D]), bv=bf(b_qkv[2 * D:]),
        bo=bf(b_o), b1=f32(b1), b2=bf(b2),
        g1=f32(g1), be1=f32(be1), g2=f32(g2), be2=f32(be2),
    )
    masks = [_make_maskT(0), _make_maskT(1)]
    in_maps = []
    for c in range(N_CORES):
        ib, h = c // 2, c % 2
        xb = x[ib]
        x_perm = xb if h == 0 else np.concatenate([xb[TQ:], xb[:TQ]], axis=0)
        in_maps.append(dict(x=np.ascontiguousarray(x_perm),
                            maskT=masks[h], **shared))
    return in_maps


def _assemble_out(results):
    out = np.empty((B, T, D), np.float32)
    for c in range(N_CORES):
        ib, h = c // 2, c % 2
        out[ib, h * TQ:(h + 1) * TQ, :] = results[c]["y"]
    return out


def kernel(x, w_qkv, b_qkv, w_o, b_o, g1, be1, g2, be2, w1, b1, w2, b2):
    in_maps = _prepare_in_maps(x, w_qkv, b_qkv, w_o, b_o, g1, be1, g2, be2,
                               w1, b1, w2, b2)
    nc = _get_program()
    res = run_bass_kernel_spmd(nc, in_maps, list(range(N_CORES)))
    return _assemble_out(res.results)


if __name__ == "__main__":
    rng = np.random.default_rng(0)
    ins = dict(
        x=rng.standard_normal((B, T, D)).astype(np.float32),
        w_qkv=(rng.standard_normal((D, 3 * D)) * 0.02).astype(np.float32),
        b_qkv=np.zeros(3 * D, np.float32),
        w_o=(rng.standard_normal((D, D)) * 0.02).astype(np.float32),
        b_o=np.zeros(D, np.float32),
        g1=np.ones(D, np.float32), be1=np.zeros(D, np.float32),
        g2=np.ones(D, np.float32), be2=np.zeros(D, np.float32),
        w1=(rng.standard_normal((D, FF)) * 0.02).astype(np.float32),
        b1=np.zeros(FF, np.float32),
        w2=(rng.standard_normal((FF, D)) * 0.02).astype(np.float32),
        b2=np.zeros(D, np.float32),
    )
    y = kernel(**ins)
    print("kernel ran, out shape", y.shape)
